# revision 23
# baseline (speedup 1.0000x reference)
"""DLRM-top kernel for 8 TRN2 NeuronCores (data-parallel over batch).

v4 — 4-sample-batched gram (FWL 128-col stationary), stride-8 FI,
interleaved emission to keep PE duty high (HAM k=8).

Per core: 4096 samples. Load/gram in half-tiles of 256; MLP in tiles of
512 (full psum bank at fp32).

  1. gpsimd cast-DMA loads x half f32->fp16 into stage [128, 64*128]
     (partition = 32*j + n for 4 samples j per group, pitch-32 junk rows).
  2. sync DMA-transpose per chunk: stage -> XT[d, 128g+32j+n].
  3. Gram: ONE matmul per 4-sample group: stationary = xt[:, 128q:+128]
     (full 128 cols -> fast weight load; junk cols produce junk psum rows
     27..31 of each 32-band, never read).  Moving streams m in
     (jj,slot,k) order, 112 cols.  Psum tile [128, 2048] = 4 banks holds
     16 groups (64 samples): group gg at col 512*(gg//4) + 112*(gg%4).
  4. Extraction psum -> FI directly: 16 copies (slot x j) per psum tile,
     4D APs, contiguous inner runs; 12 on DVE, 4 on ACT.
     FI layout fi[32*slot+n, 8*s + kt] fp16 (stride-8, 16B-aligned
     moving reads for L0); slot kt=7 holds F0 (bottom embedding row),
     copied per half from XT by gpsimd.  FI zeroed once at startup.
  5. Feature-major MLP, batch 512, fp16 weights; relu+bias: L0/L2 on
     ACT, L1/L3 on DVE tensor_scalar(add,max).  L0 = 8 k-tiles all read
     fi at stride 8 (kt'=7 -> f0 block of merged w0).
Emission interleaves mlp(t) block-units with front(2t+2)/front(2t+3)
units (which write the other fi set) so the tensor queue mixes fat MLP
streams with gram bursts -> no low-duty solo phases (HAM throttle).
"""

import sys

sys.path.insert(0, "/opt/trn_rl_repo")

import numpy as np

_B = 32768
_NCORES = 8
_BC = _B // _NCORES  # 4096
_NB = 512  # MLP batch tile
_NH = 256  # load/gram half-tile
_G = _NH // 4  # groups per half (64)
_N = 27
_D = 128
_NKT = 7
_GPT = 16  # groups per 4-bank psum tile

_compiled = {}


def _build(nt):
    import concourse.bass as bass
    import concourse.mybir as mybir
    import concourse.tile as tile
    from concourse import bacc

    f16 = mybir.dt.float16
    f32 = mybir.dt.float32
    Relu = mybir.ActivationFunctionType.Relu
    Copy = mybir.ActivationFunctionType.Copy
    Add = mybir.AluOpType.add
    Max = mybir.AluOpType.max

    nh = 2 * nt
    nb = _NB
    g = _G

    nc = bacc.Bacc("TRN2", target_bir_lowering=False, debug=False,
                   num_devices=_NCORES)

    x = nc.dram_tensor("x", [nh * _NH * _N * _D + 8 * _D], f32,
                       kind="ExternalInput")
    y = nc.dram_tensor("y", [nb * nt, 1], f32, kind="ExternalOutput")
    w0all = nc.dram_tensor("w0all", [128, 8 * 1024], f16, kind="ExternalInput")
    w1T = nc.dram_tensor("w1T", [128, 8 * 1024], f16, kind="ExternalInput")
    w2T = nc.dram_tensor("w2T", [128, 8 * 512], f16, kind="ExternalInput")
    w3T = nc.dram_tensor("w3T", [128, 4 * 256], f16, kind="ExternalInput")
    w4T = nc.dram_tensor("w4T", [128, 2], f16, kind="ExternalInput")
    b0 = nc.dram_tensor("b0", [128, 8], f32, kind="ExternalInput")
    b1 = nc.dram_tensor("b1", [128, 8], f32, kind="ExternalInput")
    b2 = nc.dram_tensor("b2", [128, 4], f32, kind="ExternalInput")
    b3 = nc.dram_tensor("b3", [128, 2], f32, kind="ExternalInput")
    b4 = nc.dram_tensor("b4", [1, 1], f32, kind="ExternalInput")

    with tile.TileContext(nc) as tc:
        import contextlib

        with contextlib.ExitStack() as ctx:
            singles = ctx.enter_context(tc.tile_pool(name="singles", bufs=1))
            stage_p = ctx.enter_context(tc.tile_pool(name="stage", bufs=2))
            xt_p = ctx.enter_context(tc.tile_pool(name="xt", bufs=3))
            act_p = ctx.enter_context(tc.tile_pool(name="act", bufs=2))
            out_p = ctx.enter_context(tc.tile_pool(name="out", bufs=2))
            ps_gram = ctx.enter_context(
                tc.tile_pool(name="psgram", bufs=1, space="PSUM"))
            ps_mlp = ctx.enter_context(
                tc.tile_pool(name="psmlp", bufs=3, space="PSUM"))
            ps_l4 = ctx.enter_context(
                tc.tile_pool(name="psl4", bufs=1, space="PSUM"))

            b0_s = singles.tile([128, 8], f32)
            nc.scalar.dma_start(out=b0_s[:], in_=b0[:])
            b1_s = singles.tile([128, 8], f32)
            nc.scalar.dma_start(out=b1_s[:], in_=b1[:])
            b2_s = singles.tile([128, 4], f32)
            nc.scalar.dma_start(out=b2_s[:], in_=b2[:])
            b3_s = singles.tile([128, 2], f32)
            nc.scalar.dma_start(out=b3_s[:], in_=b3[:])
            b4_s = singles.tile([1, 1], f32)
            nc.scalar.dma_start(out=b4_s[:], in_=b4[:])
            w4T_s = singles.tile([128, 2], f16)
            nc.scalar.dma_start(out=w4T_s[:], in_=w4T[:])
            w0_s = singles.tile([128, 8 * 1024], f16)
            nc.scalar.dma_start(out=w0_s[:], in_=w0all[:])
            w1T_s = singles.tile([128, 8 * 1024], f16)
            nc.sync.dma_start(out=w1T_s[:], in_=w1T[:])

            fi_sets = []
            fk_sets = []
            for si in range(2):
                fi = singles.tile([128, nb * 8], f16, name=f"fi{si}",
                                  tag=f"fi{si}")
                nc.vector.memset(fi[:], 0.0)
                fi_sets.append(fi)
                fk = singles.tile([128, nb * 8], f16, name=f"fk{si}",
                                  tag=f"fk{si}")
                nc.vector.memset(fk[:], 0.0)
                fk_sets.append(fk)

            xap = x[:]

            def load(h, chunks):
                stage = stage_p.tile([128, g * 128], f16, tag="stage")
                xt3 = xt_p.tile([128, g, 128], f16, tag="xt")
                g0 = 0
                for gc in chunks:
                    for j in range(4):
                        src = bass.AP(
                            tensor=xap.tensor,
                            offset=(h * _NH + g0 * 4 + j) * _N * _D,
                            ap=[[_D, 32], [4 * _N * _D, gc], [1, _D]],
                        )
                        nc.gpsimd.dma_start(
                            out=stage[32 * j:32 * j + 32,
                                      g0 * 128:(g0 + gc) * 128],
                            in_=src)
                    nc.sync.dma_start(out=xt3[:, g0:g0 + gc, :],
                                      in_=stage[:, g0 * 128:(g0 + gc) * 128],
                                      transpose=True)
                    g0 += gc
                return xt3

            ecount = [0]

            def front_units(h, xt3):
                fi = fi_sets[(h // 2) % 2]
                cb = (h % 2) * _NH
                xt = xt3[:].rearrange("p g c -> p (g c)")

                def u_f0():
                    # F0 (n=0 row) of each sample -> fi[:, 8*s + 7]
                    src = bass.AP(tensor=xt.tensor, offset=xt.offset,
                                  ap=[xt.ap[0], [128, g], [32, 4]])
                    fsl = fi[:, cb * 8 + 7:cb * 8 + 8]
                    dst = bass.AP(tensor=fsl.tensor, offset=fsl.offset,
                                  ap=[fsl.ap[0], [32, g], [8, 4]])
                    nc.gpsimd.tensor_copy(dst, src)

                units = [u_f0]
                for bt in range(g // _GPT):  # 4 psum tiles per half

                    def u_bank(bt=bt):
                        gps = ps_gram.tile([128, 2048], f32)
                        for gg in range(_GPT):
                            q = bt * _GPT + gg
                            c0 = q * 128
                            oc = 512 * (gg // 4) + 112 * (gg % 4)
                            xsl = xt[:, c0:c0 + 128]
                            mov = bass.AP(
                                tensor=xsl.tensor, offset=xsl.offset,
                                ap=[xsl.ap[0], [32, 4], [1, 4], [4, 7]])
                            nc.tensor.matmul(
                                gps[0:128, oc:oc + 112],
                                xsl, mov, start=True, stop=True)
                        for slot in range(4):
                            kk = 7 if slot < 2 else 6
                            for j in range(4):
                                gsl = gps[32 * j:32 * j + _N,
                                          28 * j + 7 * slot:
                                          28 * j + 7 * slot + kk]
                                src = bass.AP(
                                    tensor=gsl.tensor, offset=gsl.offset,
                                    ap=[gsl.ap[0], [512, 4], [112, 4],
                                        [1, kk]],
                                )
                                fc = (cb + 64 * bt + j) * 8
                                fsl = fi[32 * slot:32 * slot + _N,
                                         fc:fc + kk]
                                dst = bass.AP(
                                    tensor=fsl.tensor, offset=fsl.offset,
                                    ap=[fsl.ap[0], [128, 4], [32, 4],
                                        [1, kk]],
                                )
                                if ecount[0] % 4 == 3:
                                    nc.scalar.activation(dst, src, Copy)
                                else:
                                    nc.vector.tensor_copy(dst, src)
                                ecount[0] += 1

                    units.append(u_bank)
                return units

            def convert_units(t):
                # fi (interleaved, stride-8) -> fk (kt-major, contiguous)
                # on gpsimd; runs late in iter t-1 while mlp(t-1) drains.
                fi = fi_sets[t % 2]
                fk = fk_sets[t % 2]

                def u_conv(kt):
                    fsl = fi[:, kt:kt + 1]
                    src = bass.AP(tensor=fsl.tensor, offset=fsl.offset,
                                  ap=[fsl.ap[0], [8, nb]])
                    nc.gpsimd.tensor_copy(fk[:, kt * nb:(kt + 1) * nb], src)

                return [lambda kt=kt: u_conv(kt) for kt in range(8)]

            a1_of = {}

            def l0_units(t):
                fk = fk_sets[t % 2]
                a1 = act_p.tile([128, 8 * nb], f16, tag="a1")
                a1_of[t] = a1

                def u_l0(m8):
                    ps = ps_mlp.tile([128, nb], f32)
                    for kt in range(8):
                        nc.tensor.matmul(
                            ps[:],
                            w0_s[:, kt * 1024 + m8 * 128:kt * 1024 + (m8 + 1) * 128],
                            fk[:, kt * nb:(kt + 1) * nb],
                            start=(kt == 0), stop=(kt == 7),
                        )
                    nc.scalar.activation(a1[:, m8 * nb:(m8 + 1) * nb], ps[:],
                                         Relu, bias=b0_s[:, m8:m8 + 1])

                return [lambda m8=m8: u_l0(m8) for m8 in range(8)]

            def rest_units(t):
                a1 = a1_of.pop(t)
                a2 = act_p.tile([128, 8 * nb], f16, tag="a2")
                a3 = act_p.tile([128, 4 * nb], f16, tag="a3")
                a4 = act_p.tile([128, 2 * nb], f16, tag="a4")
                units = []

                def u_l1(m8):
                    ps = ps_mlp.tile([128, nb], f32)
                    for ko in range(8):
                        nc.tensor.matmul(
                            ps[:],
                            w1T_s[:, ko * 1024 + m8 * 128:ko * 1024 + (m8 + 1) * 128],
                            a1[:, ko * nb:(ko + 1) * nb],
                            start=(ko == 0), stop=(ko == 7),
                        )
                    nc.vector.tensor_scalar(a2[:, m8 * nb:(m8 + 1) * nb],
                                            ps[:], b1_s[:, m8:m8 + 1], 0.0,
                                            Add, Max)

                def u_l2(m4):
                    ps = ps_mlp.tile([128, nb], f32)
                    for ko in range(8):
                        nc.tensor.matmul(
                            ps[:],
                            w2T_s[:, ko * 512 + m4 * 128:ko * 512 + (m4 + 1) * 128],
                            a2[:, ko * nb:(ko + 1) * nb],
                            start=(ko == 0), stop=(ko == 7),
                        )
                    nc.scalar.activation(a3[:, m4 * nb:(m4 + 1) * nb], ps[:],
                                         Relu, bias=b2_s[:, m4:m4 + 1])

                def u_l3(m2):
                    ps = ps_mlp.tile([128, nb], f32)
                    for ko in range(4):
                        nc.tensor.matmul(
                            ps[:],
                            w3T_s[:, ko * 256 + m2 * 128:ko * 256 + (m2 + 1) * 128],
                            a3[:, ko * nb:(ko + 1) * nb],
                            start=(ko == 0), stop=(ko == 3),
                        )
                    nc.vector.tensor_scalar(a4[:, m2 * nb:(m2 + 1) * nb],
                                            ps[:], b3_s[:, m2:m2 + 1], 0.0,
                                            Add, Max)

                def u_l4():
                    ps4 = ps_l4.tile([1, nb], f32)
                    nc.tensor.matmul(ps4[:], w4T_s[:, 0:1], a4[:, 0:nb],
                                     start=True, stop=False)
                    nc.tensor.matmul(ps4[:], w4T_s[:, 1:2], a4[:, nb:2 * nb],
                                     start=False, stop=True)
                    ov = out_p.tile([1, nb], f32)
                    nc.vector.tensor_scalar_add(ov[:], ps4[:], b4_s[0:1, 0:1])
                    nc.gpsimd.dma_start(out=y[t * nb:(t + 1) * nb, :],
                                        in_=ov[:])

                for m8 in range(8):
                    units.append(lambda m8=m8: u_l1(m8))
                for m4 in range(4):
                    units.append(lambda m4=m4: u_l2(m4))
                for m2 in range(2):
                    units.append(lambda m2=m2: u_l3(m2))
                units.append(u_l4)
                return units

            # --- prologue: fronts for tiles 0 and 1 (halves 0-3) ---
            xts = {0: load(0, [8, 8, 16, 32]), 1: load(1, [16, 16, 32]),
                   2: load(2, [32, 32])}
            for u in front_units(0, xts.pop(0)):
                u()
            w2T_s = singles.tile([128, 8 * 512], f16)
            nc.scalar.dma_start(out=w2T_s[:], in_=w2T[:])
            w3T_s = singles.tile([128, 4 * 256], f16)
            nc.scalar.dma_start(out=w3T_s[:], in_=w3T[:])
            xts[3] = load(3, [64])
            for u in front_units(1, xts.pop(1)):
                u()
            for u in convert_units(0):
                u()
            xts[4] = load(4, [g])
            for u in front_units(2, xts.pop(2)):
                u()
            xts[5] = load(5, [g])
            for u in front_units(3, xts.pop(3)):
                u()
            for u in l0_units(0):
                u()

            for t in range(nt):
                if t + 1 < nt:
                    for u in convert_units(t + 1):
                        u()
                for h in (2 * t + 6, 2 * t + 7):
                    if h < nh:
                        xts[h] = load(h, [g])
                fu = []
                for h in (2 * t + 4, 2 * t + 5):
                    if h < nh:
                        fu.extend(front_units(h, xts.pop(h)))
                # weave next tile's L0 units among this tile's L1..L4
                ru = rest_units(t)
                lu = l0_units(t + 1) if t + 1 < nt else []
                mu = []
                li = 0
                for mi, u in enumerate(ru):
                    mu.append(u)
                    if mi % 2 == 1 and li < len(lu):
                        mu.append(lu[li])
                        li += 1
                mu.extend(lu[li:])
                # interleave: 1 front unit after every 2 units
                fi_i = 0
                for mi, u in enumerate(mu):
                    u()
                    if mi % 2 == 1 and fi_i < len(fu):
                        fu[fi_i]()
                        fi_i += 1
                while fi_i < len(fu):
                    fu[fi_i]()
                    fi_i += 1

    nc.compile()
    return nc


def _prep_weights(W0, b0, W1, b1, W2, b2, W3, b3, W4, b4):
    f16 = np.float16
    tr, tc_ = np.tril_indices(_N, k=-1)
    w0all = np.zeros((128, 8 * 1024), dtype=f16)
    for p, (n, m) in enumerate(zip(tr, tc_)):
        kt, slot = m // 4, m % 4
        w0all[slot * 32 + n, kt * 1024:(kt + 1) * 1024] = W0[:, 128 + p].astype(f16)
    w0all[:, 7 * 1024:8 * 1024] = np.ascontiguousarray(W0[:, :128].T).astype(f16)

    def pack(WT, mdim, ktiles):
        K, M = WT.shape
        return (WT.reshape(ktiles, 128, M).transpose(1, 0, 2)
                .reshape(128, ktiles * M).astype(f16))

    return {
        "w0all": w0all,
        "w1T": pack(W1.T, 1024, 8),
        "w2T": pack(W2.T, 512, 8),
        "w3T": pack(W3.T, 256, 4),
        "w4T": pack(W4.T, 1, 2),
        "b0": np.ascontiguousarray(b0.reshape(8, 128).T).astype(np.float32),
        "b1": np.ascontiguousarray(b1.reshape(8, 128).T).astype(np.float32),
        "b2": np.ascontiguousarray(b2.reshape(4, 128).T).astype(np.float32),
        "b3": np.ascontiguousarray(b3.reshape(2, 128).T).astype(np.float32),
        "b4": np.array([[b4[0]]], dtype=np.float32),
    }


def kernel(**inputs):
    from concourse.bass_utils import run_bass_kernel_spmd

    x = np.asarray(inputs["bottom_output"], dtype=np.float32)
    B = x.shape[0]
    bc = B // _NCORES
    nt = bc // _NB
    key = nt
    if key not in _compiled:
        _compiled[key] = _build(nt)
    nc = _compiled[key]

    wmap = _prep_weights(
        np.asarray(inputs["W0"]), np.asarray(inputs["b0"]),
        np.asarray(inputs["W1"]), np.asarray(inputs["b1"]),
        np.asarray(inputs["W2"]), np.asarray(inputs["b2"]),
        np.asarray(inputs["W3"]), np.asarray(inputs["b3"]),
        np.asarray(inputs["W4"]), np.asarray(inputs["b4"]),
    )

    in_maps = []
    for i in range(_NCORES):
        shard = x[i * bc:(i + 1) * bc]
        xflat = np.concatenate(
            [shard.reshape(-1), np.zeros(8 * _D, dtype=np.float32)])
        m = {"x": xflat}
        m.update(wmap)
        in_maps.append(m)

    res = run_bass_kernel_spmd(nc, in_maps, list(range(_NCORES)))
    out = np.concatenate([res.results[i]["y"] for i in range(_NCORES)], axis=0)
    return out.astype(np.float32)


# revision 24
# speedup vs baseline: 1.1237x; 1.1237x over previous
"""DLRM-top kernel for 8 TRN2 NeuronCores (data-parallel over batch).

v4 — 4-sample-batched gram (FWL 128-col stationary), stride-8 FI,
interleaved emission to keep PE duty high (HAM k=8).

Per core: 4096 samples. Load/gram in half-tiles of 256; MLP in tiles of
512 (full psum bank at fp32).

  1. gpsimd cast-DMA loads x half f32->fp16 into stage [128, 64*128]
     (partition = 32*j + n for 4 samples j per group, pitch-32 junk rows).
  2. sync DMA-transpose per chunk: stage -> XT[d, 128g+32j+n].
  3. Gram: ONE matmul per 4-sample group: stationary = xt[:, 128q:+128]
     (full 128 cols -> fast weight load; junk cols produce junk psum rows
     27..31 of each 32-band, never read).  Moving streams m in
     (jj,slot,k) order, 112 cols.  Psum tile [128, 2048] = 4 banks holds
     16 groups (64 samples): group gg at col 512*(gg//4) + 112*(gg%4).
  4. Extraction psum -> FI directly: 16 copies (slot x j) per psum tile,
     4D APs, contiguous inner runs; 12 on DVE, 4 on ACT.
     FI layout fi[32*slot+n, 8*s + kt] fp16 (stride-8, 16B-aligned
     moving reads for L0); slot kt=7 holds F0 (bottom embedding row),
     copied per half from XT by gpsimd.  FI zeroed once at startup.
  5. Feature-major MLP, batch 512, fp16 weights; relu+bias: L0/L2 on
     ACT, L1/L3 on DVE tensor_scalar(add,max).  L0 = 8 k-tiles all read
     fi at stride 8 (kt'=7 -> f0 block of merged w0).
Emission interleaves mlp(t) block-units with front(2t+2)/front(2t+3)
units (which write the other fi set) so the tensor queue mixes fat MLP
streams with gram bursts -> no low-duty solo phases (HAM throttle).
"""

import sys

sys.path.insert(0, "/opt/trn_rl_repo")

import numpy as np

_B = 32768
_NCORES = 8
_BC = _B // _NCORES  # 4096
_NB = 512  # MLP batch tile
_NH = 256  # load/gram half-tile
_G = _NH // 4  # groups per half (64)
_N = 27
_D = 128
_NKT = 7
_GPT = 16  # groups per 4-bank psum tile

_compiled = {}


def _build(nt):
    import concourse.bass as bass
    import concourse.mybir as mybir
    import concourse.tile as tile
    from concourse import bacc

    f16 = mybir.dt.float16
    f32 = mybir.dt.float32
    Relu = mybir.ActivationFunctionType.Relu
    Copy = mybir.ActivationFunctionType.Copy
    Add = mybir.AluOpType.add
    Max = mybir.AluOpType.max

    nh = 2 * nt
    nb = _NB
    g = _G

    nc = bacc.Bacc("TRN2", target_bir_lowering=False, debug=False,
                   num_devices=_NCORES)

    x = nc.dram_tensor("x", [nh * _NH * _N * _D + 8 * _D], f32,
                       kind="ExternalInput")
    y = nc.dram_tensor("y", [nb * nt, 1], f32, kind="ExternalOutput")
    w0all = nc.dram_tensor("w0all", [128, 8 * 1024], f16, kind="ExternalInput")
    w1T = nc.dram_tensor("w1T", [128, 8 * 1024], f16, kind="ExternalInput")
    w2T = nc.dram_tensor("w2T", [128, 8 * 512], f16, kind="ExternalInput")
    w3T = nc.dram_tensor("w3T", [128, 4 * 256], f16, kind="ExternalInput")
    w4T = nc.dram_tensor("w4T", [128, 2], f16, kind="ExternalInput")
    b0 = nc.dram_tensor("b0", [128, 8], f32, kind="ExternalInput")
    b1 = nc.dram_tensor("b1", [128, 8], f32, kind="ExternalInput")
    b2 = nc.dram_tensor("b2", [128, 4], f32, kind="ExternalInput")
    b3 = nc.dram_tensor("b3", [128, 2], f32, kind="ExternalInput")
    b4 = nc.dram_tensor("b4", [1, 1], f32, kind="ExternalInput")

    with tile.TileContext(nc) as tc:
        import contextlib

        with contextlib.ExitStack() as ctx:
            singles = ctx.enter_context(tc.tile_pool(name="singles", bufs=1))
            stage_p = ctx.enter_context(tc.tile_pool(name="stage", bufs=2))
            xt_p = ctx.enter_context(tc.tile_pool(name="xt", bufs=3))
            act_p = ctx.enter_context(tc.tile_pool(name="act", bufs=2))
            out_p = ctx.enter_context(tc.tile_pool(name="out", bufs=2))
            ps_gram = ctx.enter_context(
                tc.tile_pool(name="psgram", bufs=1, space="PSUM"))
            ps_mlp = ctx.enter_context(
                tc.tile_pool(name="psmlp", bufs=3, space="PSUM"))
            ps_l4 = ctx.enter_context(
                tc.tile_pool(name="psl4", bufs=1, space="PSUM"))

            b0_s = singles.tile([128, 8], f32)
            nc.scalar.dma_start(out=b0_s[:], in_=b0[:])
            b1_s = singles.tile([128, 8], f32)
            nc.scalar.dma_start(out=b1_s[:], in_=b1[:])
            b2_s = singles.tile([128, 4], f32)
            nc.scalar.dma_start(out=b2_s[:], in_=b2[:])
            b3_s = singles.tile([128, 2], f32)
            nc.scalar.dma_start(out=b3_s[:], in_=b3[:])
            b4_s = singles.tile([1, 1], f32)
            nc.scalar.dma_start(out=b4_s[:], in_=b4[:])
            w4T_s = singles.tile([128, 2], f16)
            nc.scalar.dma_start(out=w4T_s[:], in_=w4T[:])
            w0_s = singles.tile([128, 8 * 1024], f16)
            nc.scalar.dma_start(out=w0_s[:], in_=w0all[:])
            w1T_s = singles.tile([128, 8 * 1024], f16)
            nc.sync.dma_start(out=w1T_s[:], in_=w1T[:])

            fi_sets = []
            fk_sets = []
            for si in range(2):
                fi = singles.tile([128, nb * 8], f16, name=f"fi{si}",
                                  tag=f"fi{si}")
                nc.vector.memset(fi[:], 0.0)
                fi_sets.append(fi)
                fk = singles.tile([128, nb * 8], f16, name=f"fk{si}",
                                  tag=f"fk{si}")
                nc.vector.memset(fk[:], 0.0)
                fk_sets.append(fk)

            xap = x[:]

            def load(h, chunks):
                stage = stage_p.tile([128, g * 128], f16, tag="stage")
                xt3 = xt_p.tile([128, g, 128], f16, tag="xt")
                g0 = 0
                for gc in chunks:
                    for j in range(4):
                        src = bass.AP(
                            tensor=xap.tensor,
                            offset=(h * _NH + g0 * 4 + j) * _N * _D,
                            ap=[[_D, 32], [4 * _N * _D, gc], [1, _D]],
                        )
                        nc.gpsimd.dma_start(
                            out=stage[32 * j:32 * j + 32,
                                      g0 * 128:(g0 + gc) * 128],
                            in_=src)
                    nc.sync.dma_start(out=xt3[:, g0:g0 + gc, :],
                                      in_=stage[:, g0 * 128:(g0 + gc) * 128],
                                      transpose=True)
                    g0 += gc
                return xt3

            ecount = [0]

            def front_units(h, xt3):
                fi = fi_sets[(h // 2) % 2]
                cb = (h % 2) * _NH
                xt = xt3[:].rearrange("p g c -> p (g c)")

                def u_f0():
                    # F0 (n=0 row) of each sample -> fi[:, 8*s + 7]
                    src = bass.AP(tensor=xt.tensor, offset=xt.offset,
                                  ap=[xt.ap[0], [128, g], [32, 4]])
                    fsl = fi[:, cb * 8 + 7:cb * 8 + 8]
                    dst = bass.AP(tensor=fsl.tensor, offset=fsl.offset,
                                  ap=[fsl.ap[0], [32, g], [8, 4]])
                    nc.gpsimd.tensor_copy(dst, src)

                units = [u_f0]
                for bt in range(g // _GPT):  # 4 psum tiles per half

                    def u_bank(bt=bt):
                        gps = ps_gram.tile([128, 2048], f32)
                        for gg in range(_GPT):
                            q = bt * _GPT + gg
                            c0 = q * 128
                            oc = 512 * (gg // 4) + 112 * (gg % 4)
                            xsl = xt[:, c0:c0 + 128]
                            mov = bass.AP(
                                tensor=xsl.tensor, offset=xsl.offset,
                                ap=[xsl.ap[0], [32, 4], [1, 4], [4, 7]])
                            nc.tensor.matmul(
                                gps[0:128, oc:oc + 112],
                                xsl, mov, start=True, stop=True)
                        for slot in range(4):
                            kk = 7 if slot < 2 else 6
                            for j in range(4):
                                gsl = gps[32 * j:32 * j + _N,
                                          28 * j + 7 * slot:
                                          28 * j + 7 * slot + kk]
                                src = bass.AP(
                                    tensor=gsl.tensor, offset=gsl.offset,
                                    ap=[gsl.ap[0], [512, 4], [112, 4],
                                        [1, kk]],
                                )
                                fc = (cb + 64 * bt + j) * 8
                                fsl = fi[32 * slot:32 * slot + _N,
                                         fc:fc + kk]
                                dst = bass.AP(
                                    tensor=fsl.tensor, offset=fsl.offset,
                                    ap=[fsl.ap[0], [128, 4], [32, 4],
                                        [1, kk]],
                                )
                                if ecount[0] % 4 == 3:
                                    nc.scalar.activation(dst, src, Copy)
                                else:
                                    nc.vector.tensor_copy(dst, src)
                                ecount[0] += 1

                    units.append(u_bank)
                return units

            def convert_units(t):
                # fi (interleaved, stride-8) -> fk (kt-major, contiguous)
                # on gpsimd; runs late in iter t-1 while mlp(t-1) drains.
                fi = fi_sets[t % 2]
                fk = fk_sets[t % 2]

                def u_conv(kt):
                    fsl = fi[:, kt:kt + 1]
                    src = bass.AP(tensor=fsl.tensor, offset=fsl.offset,
                                  ap=[fsl.ap[0], [8, nb]])
                    nc.gpsimd.tensor_copy(fk[:, kt * nb:(kt + 1) * nb], src)

                return [lambda kt=kt: u_conv(kt) for kt in range(8)]

            a1_of = {}

            def l0_units(t):
                fk = fk_sets[t % 2]
                a1 = act_p.tile([128, 8 * nb], f16, tag="a1")
                a1_of[t] = a1

                def u_l0(m8):
                    ps = ps_mlp.tile([128, nb], f32)
                    for kt in range(8):
                        nc.tensor.matmul(
                            ps[:],
                            w0_s[:, kt * 1024 + m8 * 128:kt * 1024 + (m8 + 1) * 128],
                            fk[:, kt * nb:(kt + 1) * nb],
                            start=(kt == 0), stop=(kt == 7),
                        )
                    nc.scalar.activation(a1[:, m8 * nb:(m8 + 1) * nb], ps[:],
                                         Relu, bias=b0_s[:, m8:m8 + 1])

                return [lambda m8=m8: u_l0(m8) for m8 in range(8)]

            def rest_units(t):
                a1 = a1_of.pop(t)
                a2 = act_p.tile([128, 8 * nb], f16, tag="a2")
                a3 = act_p.tile([128, 4 * nb], f16, tag="a3")
                a4 = act_p.tile([128, 2 * nb], f16, tag="a4")
                units = []

                def u_l1(m8):
                    ps = ps_mlp.tile([128, nb], f32)
                    for ko in range(8):
                        nc.tensor.matmul(
                            ps[:],
                            w1T_s[:, ko * 1024 + m8 * 128:ko * 1024 + (m8 + 1) * 128],
                            a1[:, ko * nb:(ko + 1) * nb],
                            start=(ko == 0), stop=(ko == 7),
                        )
                    nc.vector.tensor_scalar(a2[:, m8 * nb:(m8 + 1) * nb],
                                            ps[:], b1_s[:, m8:m8 + 1], 0.0,
                                            Add, Max)

                def u_l2(m4):
                    ps = ps_mlp.tile([128, nb], f32)
                    for ko in range(8):
                        nc.tensor.matmul(
                            ps[:],
                            w2T_s[:, ko * 512 + m4 * 128:ko * 512 + (m4 + 1) * 128],
                            a2[:, ko * nb:(ko + 1) * nb],
                            start=(ko == 0), stop=(ko == 7),
                        )
                    nc.scalar.activation(a3[:, m4 * nb:(m4 + 1) * nb], ps[:],
                                         Relu, bias=b2_s[:, m4:m4 + 1])

                def u_l3(m2):
                    ps = ps_mlp.tile([128, nb], f32)
                    for ko in range(4):
                        nc.tensor.matmul(
                            ps[:],
                            w3T_s[:, ko * 256 + m2 * 128:ko * 256 + (m2 + 1) * 128],
                            a3[:, ko * nb:(ko + 1) * nb],
                            start=(ko == 0), stop=(ko == 3),
                        )
                    nc.vector.tensor_scalar(a4[:, m2 * nb:(m2 + 1) * nb],
                                            ps[:], b3_s[:, m2:m2 + 1], 0.0,
                                            Add, Max)

                def u_l4():
                    ps4 = ps_l4.tile([1, nb], f32)
                    nc.tensor.matmul(ps4[:], w4T_s[:, 0:1], a4[:, 0:nb],
                                     start=True, stop=False)
                    nc.tensor.matmul(ps4[:], w4T_s[:, 1:2], a4[:, nb:2 * nb],
                                     start=False, stop=True)
                    ov = out_p.tile([1, nb], f32)
                    nc.vector.tensor_scalar_add(ov[:], ps4[:], b4_s[0:1, 0:1])
                    nc.gpsimd.dma_start(out=y[t * nb:(t + 1) * nb, :],
                                        in_=ov[:])

                for m8 in range(8):
                    units.append(lambda m8=m8: u_l1(m8))
                for m4 in range(4):
                    units.append(lambda m4=m4: u_l2(m4))
                for m2 in range(2):
                    units.append(lambda m2=m2: u_l3(m2))
                units.append(u_l4)
                return units

            # --- prologue: fronts for tiles 0 and 1 (halves 0-3) ---
            xts = {0: load(0, [8, 8, 16, 32]), 1: load(1, [16, 16, 32]),
                   2: load(2, [32, 32])}
            for u in front_units(0, xts.pop(0)):
                u()
            w2T_s = singles.tile([128, 8 * 512], f16)
            nc.scalar.dma_start(out=w2T_s[:], in_=w2T[:])
            w3T_s = singles.tile([128, 4 * 256], f16)
            nc.scalar.dma_start(out=w3T_s[:], in_=w3T[:])
            xts[3] = load(3, [64])
            for u in front_units(1, xts.pop(1)):
                u()
            for u in convert_units(0):
                u()
            xts[4] = load(4, [g])
            for u in front_units(2, xts.pop(2)):
                u()
            xts[5] = load(5, [g])
            for u in front_units(3, xts.pop(3)):
                u()
            for u in l0_units(0):
                u()

            for t in range(nt):
                if t + 1 < nt:
                    for u in convert_units(t + 1):
                        u()
                for h in (2 * t + 6, 2 * t + 7):
                    if h < nh:
                        xts[h] = load(h, [g])
                fu = []
                for h in (2 * t + 4, 2 * t + 5):
                    if h < nh:
                        fu.extend(front_units(h, xts.pop(h)))
                # weave next tile's L0 units among this tile's L2/L3/L4
                # tail (converts(t+1) are done by then; L1 phase stays
                # unblocked at iteration start)
                ru = rest_units(t)
                lu = l0_units(t + 1) if t + 1 < nt else []
                mu = list(ru[:8])
                li = 0
                for u in ru[8:]:
                    mu.append(u)
                    if li < len(lu):
                        mu.append(lu[li])
                        li += 1
                mu.extend(lu[li:])
                # interleave: 1 front unit after every 2 units
                fi_i = 0
                for mi, u in enumerate(mu):
                    u()
                    if mi % 2 == 1 and fi_i < len(fu):
                        fu[fi_i]()
                        fi_i += 1
                while fi_i < len(fu):
                    fu[fi_i]()
                    fi_i += 1

    nc.compile()
    return nc


def _prep_weights(W0, b0, W1, b1, W2, b2, W3, b3, W4, b4):
    f16 = np.float16
    tr, tc_ = np.tril_indices(_N, k=-1)
    w0all = np.zeros((128, 8 * 1024), dtype=f16)
    for p, (n, m) in enumerate(zip(tr, tc_)):
        kt, slot = m // 4, m % 4
        w0all[slot * 32 + n, kt * 1024:(kt + 1) * 1024] = W0[:, 128 + p].astype(f16)
    w0all[:, 7 * 1024:8 * 1024] = np.ascontiguousarray(W0[:, :128].T).astype(f16)

    def pack(WT, mdim, ktiles):
        K, M = WT.shape
        return (WT.reshape(ktiles, 128, M).transpose(1, 0, 2)
                .reshape(128, ktiles * M).astype(f16))

    return {
        "w0all": w0all,
        "w1T": pack(W1.T, 1024, 8),
        "w2T": pack(W2.T, 512, 8),
        "w3T": pack(W3.T, 256, 4),
        "w4T": pack(W4.T, 1, 2),
        "b0": np.ascontiguousarray(b0.reshape(8, 128).T).astype(np.float32),
        "b1": np.ascontiguousarray(b1.reshape(8, 128).T).astype(np.float32),
        "b2": np.ascontiguousarray(b2.reshape(4, 128).T).astype(np.float32),
        "b3": np.ascontiguousarray(b3.reshape(2, 128).T).astype(np.float32),
        "b4": np.array([[b4[0]]], dtype=np.float32),
    }


def kernel(**inputs):
    from concourse.bass_utils import run_bass_kernel_spmd

    x = np.asarray(inputs["bottom_output"], dtype=np.float32)
    B = x.shape[0]
    bc = B // _NCORES
    nt = bc // _NB
    key = nt
    if key not in _compiled:
        _compiled[key] = _build(nt)
    nc = _compiled[key]

    wmap = _prep_weights(
        np.asarray(inputs["W0"]), np.asarray(inputs["b0"]),
        np.asarray(inputs["W1"]), np.asarray(inputs["b1"]),
        np.asarray(inputs["W2"]), np.asarray(inputs["b2"]),
        np.asarray(inputs["W3"]), np.asarray(inputs["b3"]),
        np.asarray(inputs["W4"]), np.asarray(inputs["b4"]),
    )

    in_maps = []
    for i in range(_NCORES):
        shard = x[i * bc:(i + 1) * bc]
        xflat = np.concatenate(
            [shard.reshape(-1), np.zeros(8 * _D, dtype=np.float32)])
        m = {"x": xflat}
        m.update(wmap)
        in_maps.append(m)

    res = run_bass_kernel_spmd(nc, in_maps, list(range(_NCORES)))
    out = np.concatenate([res.results[i]["y"] for i in range(_NCORES)], axis=0)
    return out.astype(np.float32)
